# revision 3
# baseline (speedup 1.0000x reference)
"""Trainium2 Bass kernel for nn_Attention_30760555774660 (stacked attention VQA).

Sharding: data-parallel over batch, 256 -> 8 cores x 32. Weights replicated.

Per-core design (B=32, S=196, D=1024, A=512, O=3000; T = B*S = 6272 tokens):
  - All matmul operands bf16 (tolerance 2e-2 allows it); f32 only in PSUM and
    for softmax accumulators.
  - Host pre-tiles every tensor into the exact SBUF layout so every DMA is a
    single contiguous-per-partition 1MB-ish transfer:
      imgT  [13, 128, 8, 512]  (d-on-partition chunks x 512-token slices)
      imgN  [13, 128, 4, 1024] (token-on-partition chunks)
  - Pass A: for each 128-token chunk, accumulate both projections
    P1 = img@W_ia1 (+ QP1 broadcast via selector matmul) and P2 = img@W_ia2
    against the same stationary imgT chunk.  tanh on ScalarE (PSUM->SBUF),
    logits via one fused DVE tensor_tensor_reduce against broadcast Wp1.
    Softmax is kept in token space, unnormalized: E = exp(logits),
    M_c = Sel^T * E (per-partition scalar mult), and vI1/Z1 accumulate over
    all 49 chunks in PSUM with M_c stationary (deferred 1/Z normalization).
    P2 parks in SBUF as bf16.
  - u1 = vI1*R1 + ques; u1T via 8 PE transposes; QP2 = u1 @ W_qa2 (+bias via
    ones-row matmul).  Pass B replays parked P2 (identity matmul into PSUM)
    + QP2 selector matmul, same logits/softmax/vI flow, u2, final FC with
    streamed W_fc and bias via ones-row matmul.
"""

import os
import sys

import numpy as np

if "/opt/trn_rl_repo" not in sys.path:
    sys.path.insert(0, "/opt/trn_rl_repo")

# Validated configuration (HW-tested). The fused InstTensorTensorReduce
# triggered a device-side failure on TRN2 in this kernel (bisected on HW);
# the split tensor_mul+tensor_reduce form is stable and only costs ~40us of
# DVE time, well under the PE-bound critical path.
ABLATE = "full"
USE_TTR = False
VI_MODE = 1  # vI/Z accumulate in one long PSUM group across all chunks
USE_VI = True

B_FULL = 256
N_CORES = 8
B = B_FULL // N_CORES  # 32
S = 196
D = 1024
A = 512
O = 3000
T = B * S  # 6272
SL = 512  # tokens per slice
NSLICE = 13  # 12 full + 1 partial (128 tokens)
TPAD = NSLICE * SL  # 6656
NCHUNK = 49  # real 128-token chunks (6272 = 49*128)
DC = D // 128  # 8
OC = 6
ON = O // OC  # 500

_nc_cache = None


def _chunks_of_slice(s):
    n = 4 if s < NSLICE - 1 else 1
    return [4 * s + k for k in range(n)]


def _build_nc():
    import concourse.bacc as bacc
    import concourse.tile as tile
    from concourse import mybir

    f32 = mybir.dt.float32
    bf16 = mybir.dt.bfloat16
    Tanh = mybir.ActivationFunctionType.Tanh
    Exp = mybir.ActivationFunctionType.Exp
    mult = mybir.AluOpType.mult
    add = mybir.AluOpType.add

    nc = bacc.Bacc("TRN2", target_bir_lowering=False)

    imgT_h = nc.dram_tensor("imgT", [NSLICE, 128, DC, SL], bf16, kind="ExternalInput")
    imgN_h = nc.dram_tensor("imgN", [NSLICE, 128, 4, D], bf16, kind="ExternalInput")
    wia1_h = nc.dram_tensor("wia1", [128, DC, A], bf16, kind="ExternalInput")
    wia2_h = nc.dram_tensor("wia2", [128, DC, A], bf16, kind="ExternalInput")
    wqa1_h = nc.dram_tensor("wqa1", [128, DC, A], bf16, kind="ExternalInput")
    wqa2_h = nc.dram_tensor("wqa2", [128, DC, A], bf16, kind="ExternalInput")
    wfc_h = nc.dram_tensor("wfc", [OC, 128, DC, ON], bf16, kind="ExternalInput")
    quesT_h = nc.dram_tensor("quesT", [128, DC, B], bf16, kind="ExternalInput")
    quesN_h = nc.dram_tensor("quesN", [B, D], bf16, kind="ExternalInput")
    sel_h = nc.dram_tensor("sel", [B, 52, 128], bf16, kind="ExternalInput")
    sct_h = nc.dram_tensor("sct", [128, 52, B], bf16, kind="ExternalInput")
    ident_h = nc.dram_tensor("ident", [128, 128], bf16, kind="ExternalInput")
    onesc_h = nc.dram_tensor("onesc", [128, 128], bf16, kind="ExternalInput")
    bqa1_h = nc.dram_tensor("bqa1", [B, A], bf16, kind="ExternalInput")
    bqa2_h = nc.dram_tensor("bqa2", [B, A], bf16, kind="ExternalInput")
    wp1_h = nc.dram_tensor("wp1", [128, A], bf16, kind="ExternalInput")
    wp2_h = nc.dram_tensor("wp2", [128, A], bf16, kind="ExternalInput")
    bfc_h = nc.dram_tensor("bfc", [B, O], bf16, kind="ExternalInput")
    score_h = nc.dram_tensor("score", [B, O], f32, kind="ExternalOutput")

    with tile.TileContext(nc) as tc:
        with (
            tc.tile_pool(name="const", bufs=1) as const,
            tc.tile_pool(name="park", bufs=1) as park,
            tc.tile_pool(name="imgt", bufs=3) as imgt_p,
            tc.tile_pool(name="imgn", bufs=3) as imgn_p,
            tc.tile_pool(name="wfcs", bufs=2) as wfc_p,
            tc.tile_pool(name="ha", bufs=4) as ha_p,
            tc.tile_pool(name="prod", bufs=2) as prod_p,
            tc.tile_pool(name="lt", bufs=2) as lt_p,
            tc.tile_pool(name="ee", bufs=2) as e_p,
            tc.tile_pool(name="mm", bufs=8) as m_p,
            tc.tile_pool(name="small", bufs=1) as small,
            tc.tile_pool(name="pp1", bufs=2, space="PSUM") as pp1_p,
            tc.tile_pool(name="pp2", bufs=2, space="PSUM") as pp2_p,
            tc.tile_pool(name="vip", bufs=3, space="PSUM") as vi_p,
            tc.tile_pool(name="wps", bufs=1, space="PSUM") as wps_p,
        ):
            # ---------------- constants ----------------
            wia1 = const.tile([128, DC, A], bf16)
            nc.sync.dma_start(out=wia1, in_=wia1_h[:, :, :])
            wia2 = const.tile([128, DC, A], bf16)
            nc.sync.dma_start(out=wia2, in_=wia2_h[:, :, :])
            wqa1 = const.tile([128, DC, A], bf16)
            nc.sync.dma_start(out=wqa1, in_=wqa1_h[:, :, :])
            wqa2 = const.tile([128, DC, A], bf16)
            nc.sync.dma_start(out=wqa2, in_=wqa2_h[:, :, :])
            quesT = const.tile([128, DC, B], bf16)
            nc.sync.dma_start(out=quesT, in_=quesT_h[:, :, :])
            quesN = const.tile([B, D], bf16)
            nc.sync.dma_start(out=quesN, in_=quesN_h[:, :])
            sel = const.tile([B, 52, 128], bf16)
            nc.sync.dma_start(out=sel, in_=sel_h[:, :, :])
            sct = const.tile([128, 52, B], bf16)
            nc.sync.dma_start(out=sct, in_=sct_h[:, :, :])
            ident = const.tile([128, 128], bf16)
            nc.sync.dma_start(out=ident, in_=ident_h[:, :])
            onesc = const.tile([128, 128], bf16)
            nc.sync.dma_start(out=onesc, in_=onesc_h[:, :])
            bqa1 = const.tile([B, A], bf16)
            nc.sync.dma_start(out=bqa1, in_=bqa1_h[:, :])
            bqa2 = const.tile([B, A], bf16)
            nc.sync.dma_start(out=bqa2, in_=bqa2_h[:, :])
            wp1b = const.tile([128, A], bf16)
            nc.sync.dma_start(out=wp1b, in_=wp1_h[:, :])
            wp2b = const.tile([128, A], bf16)
            nc.sync.dma_start(out=wp2b, in_=wp2_h[:, :])
            bfc = const.tile([B, O], bf16)
            nc.sync.dma_start(out=bfc, in_=bfc_h[:, :])

            p2park = park.tile([128, NCHUNK, A], bf16)

            # ---------------- QP1 = ques @ W_qa1 + b_qa1 ----------------
            qp_ps = wps_p.tile([B, A], f32, tag="wps")
            for dc in range(DC):
                nc.tensor.matmul(
                    qp_ps, quesT[:, dc, :], wqa1[:, dc, :],
                    start=(dc == 0), stop=(dc == DC - 1),
                )
            QP1 = small.tile([B, A], bf16, tag="qp1")
            nc.vector.tensor_add(QP1, qp_ps, bqa1)

            # ---------------- attention pass over token chunks ----------------
            def attn_pass(blk, wia_a, wia_b, wpb, QP, vi_a, vi_b, z_ps):
                """One pass over all token slices.  blk=1: compute P1 (fresh
                projections, park P2), blk=2: replay parked P2.
                Fills vi_a/vi_b/z_ps (PSUM when VI_MODE==1, SBUF f32 when 2)."""
                pend = None  # (slice, [M tiles], imgn tile, chunk list)

                def emit_vi(p):
                    if not USE_VI:
                        return
                    ps, Ms, inb, chs = p
                    if VI_MODE == 2:
                        # per-slice psum group, accumulated into SBUF f32
                        pa = vi_p.tile([B, A], f32, tag="vi")
                        pb = vi_p.tile([B, A], f32, tag="vi")
                        pz = vi_p.tile([B, 1], f32, tag="vi")
                        nj = len(chs)
                        for j, c in enumerate(chs):
                            k = c - 4 * ps
                            nc.tensor.matmul(
                                pa, Ms[j], inb[:, k, 0:A],
                                start=(j == 0), stop=(j == nj - 1),
                            )
                            nc.tensor.matmul(
                                pb, Ms[j], inb[:, k, A:D],
                                start=(j == 0), stop=(j == nj - 1),
                            )
                            nc.tensor.matmul(
                                pz, Ms[j], onesc[:, 0:1],
                                start=(j == 0), stop=(j == nj - 1),
                            )
                        nc.vector.tensor_add(vi_a, vi_a, pa)
                        nc.vector.tensor_add(vi_b, vi_b, pb)
                        nc.vector.tensor_add(z_ps, z_ps, pz)
                        return
                    for j, c in enumerate(chs):
                        k = c - 4 * ps
                        first = c == 0
                        last = c == NCHUNK - 1
                        nc.tensor.matmul(
                            vi_a, Ms[j], inb[:, k, 0:A],
                            start=first, stop=last,
                        )
                        nc.tensor.matmul(
                            vi_b, Ms[j], inb[:, k, A:D],
                            start=first, stop=last,
                        )
                        nc.tensor.matmul(
                            z_ps, Ms[j], onesc[:, 0:1],
                            start=first, stop=last,
                        )

                for s in range(NSLICE):
                    chs = _chunks_of_slice(s)
                    if blk == 1:
                        itb = imgt_p.tile([128, DC, SL], bf16, tag="imgt")
                        nc.sync.dma_start(out=itb, in_=imgT_h[s, :, :, :])
                    inb = imgn_p.tile([128, 4, D], bf16, tag="imgn")
                    nc.sync.dma_start(out=inb, in_=imgN_h[s, :, :, :])
                    lts = lt_p.tile([128, 4], f32, tag="lt")
                    Ms = []
                    for j, c in enumerate(chs):
                        k = c - 4 * s
                        pp = (pp1_p if (blk == 1 or c % 2 == 0) else pp2_p).tile(
                            [128, A], f32, tag="pp"
                        )
                        if blk == 1:
                            pp2 = pp2_p.tile([128, A], f32, tag="pp")
                            for dc in range(DC):
                                nc.tensor.matmul(
                                    pp, itb[:, dc, k * 128 : (k + 1) * 128],
                                    wia_a[:, dc, :],
                                    start=(dc == 0), stop=False,
                                   
                                )
                                nc.tensor.matmul(
                                    pp2, itb[:, dc, k * 128 : (k + 1) * 128],
                                    wia_b[:, dc, :],
                                    start=(dc == 0), stop=(dc == DC - 1),
                                   
                                )
                            nc.tensor.matmul(
                                pp, sel[:, c, :], QP, start=False, stop=True,
                               
                            )
                            nc.any.tensor_copy(p2park[:, c, :], pp2)
                        else:
                            nc.tensor.matmul(
                                pp, ident, p2park[:, c, :],
                                start=True, stop=False,
                            )
                            nc.tensor.matmul(
                                pp, sel[:, c, :], QP, start=False, stop=True,
                            )
                        ha = ha_p.tile([128, A], bf16, tag="ha")
                        nc.scalar.activation(ha, pp, Tanh)
                        prod = prod_p.tile([128, A], bf16, tag="prod")
                        if USE_TTR:
                            nc.vector.tensor_tensor_reduce(
                                out=prod, in0=ha, in1=wpb, scale=1.0, scalar=0.0,
                                op0=mult, op1=add, accum_out=lts[:, j : j + 1],
                            )
                        else:
                            nc.vector.tensor_mul(prod, ha, wpb)
                            nc.vector.tensor_reduce(
                                lts[:, j : j + 1], prod,
                                axis=mybir.AxisListType.X, op=add,
                            )
                        if j == 0 and pend is not None:
                            emit_vi(pend)
                    ee = e_p.tile([128, 4], f32, tag="ee")
                    nc.scalar.activation(ee[:, 0 : len(chs)], lts[:, 0 : len(chs)], Exp)
                    for j, c in enumerate(chs):
                        M = m_p.tile([128, B], bf16, tag="m")
                        nc.vector.tensor_scalar_mul(M, sct[:, c, :], ee[:, j : j + 1])
                        Ms.append(M)
                    pend = (s, Ms, inb, chs)
                emit_vi(pend)

            def vi_tiles():
                if VI_MODE == 2:
                    a = small.tile([B, A], f32, tag="via")
                    b = small.tile([B, A], f32, tag="vib")
                    z = small.tile([B, 1], f32, tag="viz")
                    nc.vector.memset(a, 0.0)
                    nc.vector.memset(b, 0.0)
                    nc.vector.memset(z, 0.0)
                    return a, b, z
                return (
                    vi_p.tile([B, A], f32, tag="vi", name="via_ps"),
                    vi_p.tile([B, A], f32, tag="vi", name="vib_ps"),
                    vi_p.tile([B, 1], f32, tag="vi", name="viz_ps"),
                )

            # ---------------- block 1 ----------------
            u1 = small.tile([B, D], bf16, tag="u1")
            if ABLATE != "noA":
                vi1a, vi1b, z1 = vi_tiles()
                attn_pass(1, wia1, wia2, wp1b, QP1, vi1a, vi1b, z1)

                r1 = small.tile([B, 1], f32, tag="r1")
                nc.vector.reciprocal(r1, z1)
                nc.vector.tensor_scalar_mul(u1[:, 0:A], vi1a, r1)
                nc.vector.tensor_scalar_mul(u1[:, A:D], vi1b, r1)
                nc.vector.tensor_add(u1, u1, quesN)
            else:
                nc.vector.tensor_copy(u1, quesN)

            u1T = small.tile([128, DC, B], bf16, tag="u1t")
            for dc in range(DC):
                pt = wps_p.tile([128, B], bf16, tag="wps")
                nc.tensor.transpose(
                    pt, u1[:, dc * 128 : (dc + 1) * 128], ident[0:B, 0:B]
                )
                nc.vector.tensor_copy(u1T[:, dc, :], pt)

            qp2_ps = wps_p.tile([B, A], f32, tag="wps")
            for dc in range(DC):
                nc.tensor.matmul(
                    qp2_ps, u1T[:, dc, :], wqa2[:, dc, :],
                    start=(dc == 0), stop=(dc == DC - 1),
                )
            QP2 = small.tile([B, A], bf16, tag="qp2")
            nc.vector.tensor_add(QP2, qp2_ps, bqa2)

            # ---------------- block 2 ----------------
            u2 = small.tile([B, D], bf16, tag="u2")
            if ABLATE == "full":
                vi2a, vi2b, z2 = vi_tiles()
                attn_pass(2, None, None, wp2b, QP2, vi2a, vi2b, z2)

                r2 = small.tile([B, 1], f32, tag="r2")
                nc.vector.reciprocal(r2, z2)
                nc.vector.tensor_scalar_mul(u2[:, 0:A], vi2a, r2)
                nc.vector.tensor_scalar_mul(u2[:, A:D], vi2b, r2)
                nc.vector.tensor_add(u2, u2, u1)
            else:
                nc.vector.tensor_copy(u2, u1)

            u2T = small.tile([128, DC, B], bf16, tag="u2t")
            for dc in range(DC):
                pt = wps_p.tile([128, B], bf16, tag="wps")
                nc.tensor.transpose(
                    pt, u2[:, dc * 128 : (dc + 1) * 128], ident[0:B, 0:B]
                )
                nc.vector.tensor_copy(u2T[:, dc, :], pt)

            # ---------------- final FC ----------------
            for n in range(OC):
                wf = wfc_p.tile([128, DC, ON], bf16, tag="wf")
                nc.sync.dma_start(out=wf, in_=wfc_h[n, :, :, :])
                fp = wps_p.tile([B, ON], f32, tag="wps")
                for dc in range(DC):
                    nc.tensor.matmul(
                        fp, u2T[:, dc, :], wf[:, dc, :],
                        start=(dc == 0), stop=(dc == DC - 1),
                    )
                sc = small.tile([B, ON], f32, tag=f"sc{n}")
                nc.vector.tensor_add(sc, fp, bfc[:, n * ON : (n + 1) * ON])
                nc.sync.dma_start(out=score_h[:, n * ON : (n + 1) * ON], in_=sc)

    nc.compile()
    return nc


def _get_nc():
    global _nc_cache
    if _nc_cache is None:
        _nc_cache = _build_nc()
    return _nc_cache


def _make_in_maps(inputs):
    import ml_dtypes

    bf16 = ml_dtypes.bfloat16

    def bf(x):
        return np.ascontiguousarray(np.asarray(x, np.float32).astype(bf16))

    # weights (shared across cores)
    def tile_dca(w):  # [D, A] -> [128, DC, A]
        return np.ascontiguousarray(
            np.asarray(w, np.float32).astype(bf16).reshape(DC, 128, A).transpose(1, 0, 2)
        )

    wfc = np.asarray(inputs["W_fc"], np.float32).astype(bf16)
    wfc_t = np.ascontiguousarray(
        wfc.reshape(DC, 128, OC, ON).transpose(2, 1, 0, 3)
    )

    # selector masks over padded token axis
    tok = np.arange(52 * 128)
    batch_of = tok // S  # >= B for pad tokens
    sel = np.zeros((B, 52 * 128), np.float32)
    valid = tok < T
    sel[batch_of[valid], tok[valid]] = 1.0
    sel_t = np.ascontiguousarray(sel.reshape(B, 52, 128).astype(bf16))
    sct_t = np.ascontiguousarray(
        sel.T.reshape(52, 128, B).transpose(1, 0, 2).astype(bf16)
    )

    ident = np.eye(128, dtype=np.float32).astype(bf16)
    onesc = np.ones((128, 128), np.float32).astype(bf16)

    shared = {
        "wia1": tile_dca(inputs["W_ia1"]),
        "wia2": tile_dca(inputs["W_ia2"]),
        "wqa1": tile_dca(inputs["W_qa1"]),
        "wqa2": tile_dca(inputs["W_qa2"]),
        "wfc": wfc_t,
        "sel": sel_t,
        "sct": sct_t,
        "ident": np.ascontiguousarray(ident),
        "onesc": np.ascontiguousarray(onesc),
        "bqa1": np.ascontiguousarray(
            np.broadcast_to(bf(inputs["b_qa1"]), (B, A))
        ),
        "bqa2": np.ascontiguousarray(
            np.broadcast_to(bf(inputs["b_qa2"]), (B, A))
        ),
        "wp1": np.ascontiguousarray(
            np.broadcast_to(bf(inputs["Wp1"]), (128, A))
        ),
        "wp2": np.ascontiguousarray(
            np.broadcast_to(bf(inputs["Wp2"]), (128, A))
        ),
        "bfc": np.ascontiguousarray(
            np.broadcast_to(bf(inputs["b_fc"]), (B, O))
        ),
    }

    in_maps = []
    for core in range(N_CORES):
        slc = slice(core * B, (core + 1) * B)
        img = np.asarray(inputs["img_feat"][slc], np.float32).astype(bf16)
        flat = img.reshape(T, D)
        flat_pad = np.zeros((TPAD, D), bf16)
        flat_pad[:T] = flat
        imgN_t = np.ascontiguousarray(
            flat_pad.reshape(NSLICE, 4, 128, D).transpose(0, 2, 1, 3)
        )
        flatT_pad = np.ascontiguousarray(flat_pad.T)  # [D, TPAD]
        imgT_t = np.ascontiguousarray(
            flatT_pad.reshape(DC, 128, NSLICE, SL).transpose(2, 1, 0, 3)
        )
        ques = np.asarray(inputs["ques_feat"][slc], np.float32).astype(bf16)
        quesT_t = np.ascontiguousarray(
            ques.T.reshape(DC, 128, B).transpose(1, 0, 2)
        )
        m = dict(shared)
        m["imgT"] = imgT_t
        m["imgN"] = imgN_t
        m["quesT"] = quesT_t
        m["quesN"] = np.ascontiguousarray(ques)
        in_maps.append(m)
    return in_maps


def kernel_run(inputs, trace=False):
    from concourse.bass_utils import run_bass_kernel_spmd

    nc = _get_nc()
    in_maps = _make_in_maps(inputs)
    res = run_bass_kernel_spmd(nc, in_maps, core_ids=list(range(N_CORES)), trace=trace)
    out = np.concatenate([r["score"] for r in res.results], axis=0)
    return out, res


def kernel(**inputs):
    out, _ = kernel_run(inputs)
    return out
